# revision 7
# baseline (speedup 1.0000x reference)
"""Trainium2 Bass kernel for the CACE message-passing GNN (nn_Cace_58291296141968).

Strategy (8 NeuronCores, SPMD):
  - Receivers are load-balanced onto 8 cores x 32 subtiles x 16 node slots
    (host-side index prep only). Edges go to the subtile of their receiver,
    padded to 256 edge slots (2 blocks of 128) per subtile.
  - Per-edge radial (bessel*cutoff), angular monomials and species embeddings
    are computed on device in [128, n_blocks*w] layout.
  - Segment sums are PE matmuls: lhsT is a per-block "weighted one-hot"
    S_w[e,(r,n)] = rc[e,r] * delta(recv_slot(e)==n), built on DVE with
    broadcast APs; PSUM accumulates per subtile in layout [(r|s')*16+n, feat].
  - The shared per-l radial transform W_rt is applied post-segsum as 4 const
    block-diagonal matmuls (RTL_l), staying on the partition axis.
  - Stage 1 is pipelined in groups of 8 subtiles: as each group's A lands,
    its B0/chi/V are computed, the group's table rows are cast to bf16 on the
    scalar engine, repacked to DRAM via HWDGE, and AllGathered as a chunked
    collective — overlapping the collective with the rest of stage 1.
  - Message-passing layer: rows of the gathered [4096, 1536] bf16 node table
    [A | V] are fetched per edge with dma_gather; msg_A uses 8 sigma-sliced
    matmuls (parity-split S_w so PSUM writes stay 32-aligned), msg_Bchi uses
    the same seg-matmul + RT with the receiver-embedding factor per node.
  - B0/B1 invariants are computed on device; host only unpermutes rows.
"""
import os
import numpy as np
from math import factorial, pi

import concourse.bacc as bacc
import concourse.bass as bass
import concourse.mybir as mybir
import concourse.tile as tile
from concourse.bass_utils import run_bass_kernel_spmd

# ---- problem constants (hardcoded; must match reference.py) ----
ZS = np.array([1, 6, 7, 8], dtype=np.int64)
NZ = 4
NAB = 3
CHAN = 9
MAX_L = 3
N_RBF = 8
N_RB = 8
CUTOFF = 5.5
MP_NORM = 1.0 / 10.0 ** 0.5
N_NODES = 4000
N_EDGES = 48000

def _make_l_list(max_l):
    lst = []
    for l in range(max_l + 1):
        for lx in range(l, -1, -1):
            for ly in range(l - lx, -1, -1):
                lst.append((lx, ly, l - lx - ly))
    return lst

L_LIST = _make_l_list(MAX_L)
N_L = len(L_LIST)                                   # 20
L_OF = np.array([sum(t) for t in L_LIST])
PREF = np.array([factorial(sum(t)) / (factorial(t[0]) * factorial(t[1]) * factorial(t[2]))
                 for t in L_LIST], dtype=np.float64)
L_RANGES = [(0, 1), (1, 4), (4, 10), (10, 20)]
# monomial build chain: (i, parent, comp) for i >= 1
_MONO_CHAIN = []
for _i in range(1, N_L):
    t = L_LIST[_i]
    for _c in range(3):
        if t[_c] > 0:
            pt = list(t); pt[_c] -= 1
            _MONO_CHAIN.append((_i, L_LIST.index(tuple(pt)), _c))
            break

NC = 8
NSUB = 32
SUBN = 16
BPS = 2
EPB = 128
CAP = BPS * EPB          # 256
NBLK = NSUB * BPS        # 64 blocks/core
NROW = NSUB * SUBN       # 512 node rows/core
TABW = 1536              # table row: 1440 A + 9 V + pad (bytes % 256 == 0)
P = 128
F32 = mybir.dt.float32
BF16 = mybir.dt.bfloat16
TDT = BF16               # table + stage-2 seg dtype
I16 = mybir.dt.int16
GRP = 8                  # subtiles per stage-1/2 pipeline group

_PROGRAM = None


# ================= host-side sharding prep (index work only) =================
def _prep(positions, shifts, atomic_numbers, edge_index):
    import heapq
    snd = np.asarray(edge_index[0]).astype(np.int64)
    rcv = np.asarray(edge_index[1]).astype(np.int64)
    an = np.asarray(atomic_numbers)
    species = np.searchsorted(ZS, an)
    indeg = np.bincount(rcv, minlength=N_NODES)
    order = np.argsort(-indeg, kind="stable")
    TS = NC * NSUB
    loads = np.zeros(TS, dtype=np.int64)
    counts = np.zeros(TS, dtype=np.int64)
    assign_sub = np.zeros(N_NODES, dtype=np.int64)
    assign_slot = np.zeros(N_NODES, dtype=np.int64)
    heap = [(0, t) for t in range(TS)]
    heapq.heapify(heap)
    for nd in order:
        pending = []
        while True:
            load, t = heapq.heappop(heap)
            if counts[t] < SUBN:
                break
            pending.append((load, t))
        assign_sub[nd] = t
        assign_slot[nd] = counts[t]
        counts[t] += 1
        loads[t] = load + indeg[nd]
        heapq.heappush(heap, (loads[t], t))
        for it in pending:
            heapq.heappush(heap, it)
    assert loads.max() <= CAP, f"subtile edge overflow: {loads.max()} > {CAP}"

    core_of = assign_sub // NSUB
    sub_of = assign_sub % NSUB
    node_row = core_of * NROW + sub_of * SUBN + assign_slot      # node -> global row
    node_of_row = np.full(NC * NROW, -1, dtype=np.int64)
    node_of_row[node_row] = np.arange(N_NODES)
    # table row in the chunked-AllGather layout: [group, core, sub%GRP, slot]
    tab_row = ((sub_of // GRP) * (NC * GRP * SUBN) + core_of * (GRP * SUBN)
               + (sub_of % GRP) * SUBN + assign_slot)

    e_sub = assign_sub[rcv]
    e_order = np.argsort(e_sub, kind="stable")
    bounds = np.searchsorted(e_sub[e_order], np.arange(TS + 1))

    pos = np.asarray(positions, dtype=np.float32)
    shf = np.asarray(shifts, dtype=np.float32)

    ES = NSUB * CAP                                              # 8192 edge slots/core
    geo = np.zeros((NC, 9, ES), dtype=np.float32)                # [comp(SxyzRxyzShxyz), slot]
    geo[:, 3:6, :] = 1.0                                         # benign pad: R=(1,1,1), S=0
    recvoh = np.zeros((NC, SUBN, ES), dtype=np.float32)
    sendrow = np.zeros((NC, ES), dtype=np.int64)
    for t in range(TS):
        c = t // NSUB; s = t % NSUB
        es = e_order[bounds[t]:bounds[t + 1]]
        k = len(es)
        base = s * CAP
        geo[c, 0:3, base:base + k] = pos[snd[es]].T
        geo[c, 3:6, base:base + k] = pos[rcv[es]].T
        geo[c, 6:9, base:base + k] = shf[es].T
        recvoh[c, assign_slot[rcv[es]], base + np.arange(k)] = 1.0
        sendrow[c, base:base + k] = tab_row[snd[es]]

    # device edge-slot layout: slot -> (blk, p) with slot = blk*128 + p
    def to_pb(a):   # [NC, ..., ES] -> [NC, 128, ..., NBLK]
        a2 = a.reshape(a.shape[:-1] + (NBLK, EPB))               # [..., NBLK, 128]
        return np.moveaxis(a2, -1, 1)                            # [NC, 128, ..., NBLK]

    geo_in = np.ascontiguousarray(to_pb(geo).reshape(NC, P, 9 * NBLK))   # [NC, 128, (comp,blk)]
    recv_in = np.ascontiguousarray(to_pb(recvoh).reshape(NC, P, SUBN * NBLK))  # [NC,128,(n,blk)]
    # gather idx: per subtile 256 slots; idx k at partition k%16 (replicated), col sub*16 + k//16
    gidx = np.zeros((NC, P, NSUB * 16), dtype=np.int16)
    for c in range(NC):
        w = sendrow[c].reshape(NSUB, 16, 16).astype(np.int16)    # [sub, k//16, k%16]
        packed = w.transpose(2, 0, 1).reshape(16, NSUB * 16)     # [k%16, (sub, k//16)]
        for g in range(8):
            gidx[c, g * 16:(g + 1) * 16, :] = packed
    # per-edge-slot sender species (pad -> 0) in device layout [NC, 128, NBLK]
    sendsp = np.zeros((NC, ES), dtype=np.int64)
    for t in range(TS):
        c = t // NSUB; s = t % NSUB
        es = e_order[bounds[t]:bounds[t + 1]]
        sendsp[c, s * CAP:s * CAP + len(es)] = species[snd[es]]
    sendsp_in = to_pb(sendsp)                                    # [NC, 128, NBLK]
    # per-node-row species (empty rows -> 0; all their uses are masked/zero)
    rowsp = np.zeros((NC, NROW), dtype=np.int64)
    msk = node_of_row >= 0
    rowsp.reshape(-1)[msk] = species[node_of_row[msk]]
    return dict(geo=geo_in, recv=recv_in, gidx=gidx, sendsp=sendsp_in, rowsp=rowsp,
                node_of_row=node_of_row, node_row=node_row)


def _consts():
    iotaN = np.tile((np.arange(P) % 16).astype(np.float32)[None, :], (P, 1))
    blkdiag = ((np.arange(P)[:, None] % 16) == (np.arange(P)[None, :] % 16)).astype(np.float32)
    prefrow = np.tile(np.repeat(PREF.astype(np.float32), CHAN)[None, :], (P, 1))       # [128,180]
    nrow = np.tile((np.arange(1, N_RBF + 1) * pi / CUTOFF).astype(np.float32)[None, :], (P, 1))
    parc = np.zeros((P, 16), dtype=np.float32)                   # [par, r] keep r where r%2==par
    for par in range(2):
        for r in range(8):
            if r % 2 == par:
                parc[:, par * 8 + r] = 1.0
    consts = np.concatenate([iotaN, blkdiag, prefrow, nrow, parc], axis=1)  # [128, 460]
    repl16 = np.zeros((8, P), dtype=np.float32)
    for p in range(P):
        repl16[p // 16, p] = 1.0
    ones1 = np.ones((1, P), dtype=np.float32)
    return consts, repl16, ones1


# ================= device program =================
def _build(sim_mode=False):
    nc = bacc.Bacc("TRN2", target_bir_lowering=False, debug=False,
                   num_devices=(1 if sim_mode else NC))
    AF = mybir.ActivationFunctionType
    OP = mybir.AluOpType

    x_geo = nc.dram_tensor("x_geo", [P, 9 * NBLK], F32, kind="ExternalInput")
    x_recv = nc.dram_tensor("x_recv", [P, SUBN * NBLK], F32, kind="ExternalInput")
    x_gidx = nc.dram_tensor("x_gidx", [P, NSUB * 16], I16, kind="ExternalInput")
    x_cons = nc.dram_tensor("x_cons", [P, 460], F32, kind="ExternalInput")
    # host-replicated weight patterns: [RTLW 32 | WT 180 | EM 96] + per-edge sender emb
    x_wpack = nc.dram_tensor("x_wpack", [P, 308], F32, kind="ExternalInput")
    x_embse = nc.dram_tensor("x_embse", [P, NBLK * NAB], F32, kind="ExternalInput")
    o_b0 = nc.dram_tensor("o_b0", [P, NSUB * 45], F32, kind="ExternalOutput")
    o_b1 = nc.dram_tensor("o_b1", [P, NSUB * 45], F32, kind="ExternalOutput")

    with tile.TileContext(nc) as tc:
        with (
            tc.tile_pool(name="persist", bufs=1) as pp,
            tc.tile_pool(name="work", bufs=2) as wp,
            tc.tile_pool(name="dram", bufs=1, space="DRAM") as dr,
        ):
            # ---------- loads ----------
            cons = pp.tile([P, 460], F32)
            nc.sync.dma_start(cons[:], x_cons[:])
            iotaN = cons[:, 0:128]
            blkdiag = cons[:, 128:256]
            prefrow = cons[:, 256:436]
            nrow = cons[:, 436:444]
            parc = cons[:, 444:460]

            geo = pp.tile([P, 9 * NBLK], F32)
            recvs = pp.tile([P, SUBN * NBLK], F32)
            gidx = pp.tile([P, NSUB * 16], I16)
            wpack = pp.tile([P, 308], F32)
            embsE = pp.tile([P, NBLK * NAB], F32)
            nc.sync.dma_start(geo[:], x_geo[:])
            nc.sync.dma_start(recvs[:], x_recv[:])
            nc.sync.dma_start(gidx[:], x_gidx[:])
            nc.sync.dma_start(wpack[:], x_wpack[:])
            nc.sync.dma_start(embsE[:], x_embse[:])
            WT = wpack[:, 32:212]
            EM = wpack[:, 212:308]

            # ---------- one-time derived: RTL_l from host-shipped W rows ----------
            rtl = []
            for l in range(MAX_L + 1):
                rtl_t = pp.tile([P, P], F32, tag=f"rtl{l}")
                rtl.append(rtl_t)
                nc.vector.tensor_tensor(
                    out=rtl_t[:].rearrange("p (s n) -> p s n", s=8),
                    in0=wpack[:, l * 8:(l + 1) * 8][:, :, None].to_broadcast([P, 8, 16]),
                    in1=blkdiag.rearrange("p (s n) -> p s n", s=8),
                    op=OP.mult)

            # ---------- per-edge base phase ----------
            D = pp.tile([P, 3 * NBLK], F32)
            nc.vector.tensor_tensor(out=D[:], in0=geo[:, 3 * NBLK:6 * NBLK],
                                    in1=geo[:, 0:3 * NBLK], op=OP.subtract)
            nc.vector.tensor_tensor(out=D[:], in0=D[:], in1=geo[:, 6 * NBLK:9 * NBLK], op=OP.add)
            sq = wp.tile([P, 3 * NBLK], F32, tag="sq")
            nc.vector.tensor_tensor(out=sq[:], in0=D[:], in1=D[:], op=OP.mult)
            r2 = wp.tile([P, NBLK], F32, tag="r2")
            nc.vector.tensor_tensor(out=r2[:], in0=sq[:, 0:NBLK], in1=sq[:, NBLK:2 * NBLK], op=OP.add)
            nc.vector.tensor_tensor(out=r2[:], in0=r2[:], in1=sq[:, 2 * NBLK:3 * NBLK], op=OP.add)
            rr = wp.tile([P, NBLK], F32, tag="rr")
            nc.scalar.activation(rr[:], r2[:], AF.Sqrt)
            rinv = pp.tile([P, NBLK], F32)
            nc.vector.reciprocal(rinv[:], rr[:])
            uu = wp.tile([P, NBLK], F32, tag="uu")
            nc.vector.tensor_scalar_mul(uu[:], rr[:], 1.0 / CUTOFF)
            U = pp.tile([P, 3 * NBLK], F32)
            nc.vector.tensor_tensor(
                out=U[:].rearrange("p (c b) -> p c b", c=3),
                in0=D[:].rearrange("p (c b) -> p c b", c=3),
                in1=rinv[:, None, :].to_broadcast([P, 3, NBLK]), op=OP.mult)
            # bessel args [128, (blk, r)] + range reduction to [-pi, pi)
            arg = wp.tile([P, NBLK * 8], F32, tag="arg")
            nc.vector.tensor_tensor(
                out=arg[:].rearrange("p (b r) -> p b r", r=8),
                in0=rr[:, :, None].to_broadcast([P, NBLK, 8]),
                in1=nrow[:, None, :].to_broadcast([P, NBLK, 8]), op=OP.mult)
            ge = wp.tile([P, NBLK * 8], F32, tag="ge")
            for thr, sub in ((4 * pi, 4 * pi), (2 * pi, 2 * pi), (pi, 2 * pi)):
                nc.vector.tensor_scalar(out=ge[:], in0=arg[:], scalar1=float(thr),
                                        scalar2=float(sub), op0=OP.is_ge, op1=OP.mult)
                nc.vector.tensor_tensor(out=arg[:], in0=arg[:], in1=ge[:], op=OP.subtract)
            sinv = wp.tile([P, NBLK * 8], F32, tag="sinv")
            nc.scalar.activation(sinv[:], arg[:], AF.Sin)
            # cutoff polynomial
            u2 = wp.tile([P, NBLK], F32, tag="u2")
            nc.vector.tensor_tensor(out=u2[:], in0=uu[:], in1=uu[:], op=OP.mult)
            a1 = wp.tile([P, NBLK], F32, tag="a1")
            nc.vector.tensor_scalar(out=a1[:], in0=uu[:], scalar1=-48.0, scalar2=28.0,
                                    op0=OP.mult, op1=OP.add)
            g21 = wp.tile([P, NBLK], F32, tag="g21")
            nc.vector.tensor_scalar_mul(g21[:], u2[:], 21.0)
            nc.vector.tensor_tensor(out=g21[:], in0=g21[:], in1=a1[:], op=OP.add)
            u6 = wp.tile([P, NBLK], F32, tag="u6")
            nc.vector.tensor_tensor(out=u6[:], in0=u2[:], in1=u2[:], op=OP.mult)
            nc.vector.tensor_tensor(out=u6[:], in0=u6[:], in1=u2[:], op=OP.mult)
            fc = wp.tile([P, NBLK], F32, tag="fc")
            nc.vector.tensor_tensor(out=fc[:], in0=u6[:], in1=g21[:], op=OP.mult)
            nc.vector.tensor_scalar(out=fc[:], in0=fc[:], scalar1=-1.0, scalar2=1.0,
                                    op0=OP.mult, op1=OP.add)
            lt = wp.tile([P, NBLK], F32, tag="lt")
            nc.vector.tensor_scalar(out=lt[:], in0=uu[:], scalar1=1.0, scalar2=None, op0=OP.is_lt)
            nc.vector.tensor_tensor(out=fc[:], in0=fc[:], in1=lt[:], op=OP.mult)
            scal = wp.tile([P, NBLK], F32, tag="scal")
            nc.vector.tensor_tensor(out=scal[:], in0=rinv[:], in1=fc[:], op=OP.mult)
            nc.vector.tensor_scalar_mul(scal[:], scal[:], float(np.sqrt(2.0 / CUTOFF)))
            rc = pp.tile([P, NBLK * 8], F32)
            nc.vector.tensor_tensor(
                out=rc[:].rearrange("p (b r) -> p b r", r=8),
                in0=sinv[:].rearrange("p (b r) -> p b r", r=8),
                in1=scal[:, :, None].to_broadcast([P, NBLK, 8]), op=OP.mult)
            # parity-masked rc: rcEO [128, (blk, par, r)]; _mp variant folds MP_NORM
            rcEO = pp.tile([P, NBLK * 16], F32)
            nc.vector.tensor_tensor(
                out=rcEO[:].rearrange("p (b q r) -> p b q r", q=2, r=8),
                in0=rc[:].rearrange("p (b r) -> p b r", r=8)[:, :, None, :].to_broadcast([P, NBLK, 2, 8]),
                in1=parc.rearrange("p (q r) -> p q r", q=2)[:, None, :, :].to_broadcast([P, NBLK, 2, 8]),
                op=OP.mult)
            rcMP = pp.tile([P, NBLK * 16], F32)
            nc.vector.tensor_scalar_mul(rcMP[:], rcEO[:], float(MP_NORM))
            # angular monomials ang [128, (blk, i)]
            ang = pp.tile([P, NBLK * N_L], F32)
            angv = ang[:].rearrange("p (b i) -> p b i", i=N_L)
            nc.vector.tensor_scalar(out=angv[:, :, 0], in0=uu[:], scalar1=0.0, scalar2=1.0,
                                    op0=OP.mult, op1=OP.add)
            for i, par, c in _MONO_CHAIN:
                nc.vector.tensor_tensor(out=angv[:, :, i], in0=angv[:, :, par],
                                        in1=U[:, c * NBLK:(c + 1) * NBLK], op=OP.mult)
            # G1 [128, (blk, i, a)]
            G1 = pp.tile([P, NBLK * N_L * NAB], F32)
            nc.vector.tensor_tensor(
                out=G1[:].rearrange("p (b i a) -> p b i a", i=N_L, a=NAB),
                in0=angv[:, :, :, None].to_broadcast([P, NBLK, N_L, NAB]),
                in1=embsE[:].rearrange("p (b a) -> p b a", a=NAB)[:, :, None, :].to_broadcast([P, NBLK, N_L, NAB]),
                op=OP.mult)

            A_all = pp.tile([P, NSUB * 180], F32)
            A1_all = pp.tile([P, NSUB * 180], F32)
            B0_all = pp.tile([P, NSUB * 45], F32)
            B1_all = pp.tile([P, NSUB * 45], F32)
            mem_all = pp.tile([P, NSUB * 180], F32)

            def build_sw1(blk):
                # unsplit f32 S_w for stage 1: [128, (r,n)]
                sw = wp.tile([P, P], F32, tag="sw1", bufs=3)
                nc.vector.tensor_tensor(
                    out=sw[:].rearrange("p (r n) -> p r n", r=8),
                    in0=recvs[:].rearrange("p (n b) -> p b n", n=SUBN)[:, blk, :][:, None, :].to_broadcast([P, 8, 16]),
                    in1=rc[:, blk * 8:(blk + 1) * 8][:, :, None].to_broadcast([P, 8, 16]),
                    op=OP.mult)
                return sw

            def build_sw(blk, dt, rcsrc):
                sw = wp.tile([P, 256], dt, tag="sw" if dt == F32 else "swb", bufs=3)
                nc.vector.tensor_tensor(
                    out=sw[:].rearrange("p (q r n) -> p q r n", q=2, r=8),
                    in0=recvs[:].rearrange("p (n b) -> p b n", n=SUBN)[:, blk, :][:, None, None, :].to_broadcast([P, 2, 8, 16]),
                    in1=rcsrc[:, blk * 16:(blk + 1) * 16].rearrange("p (q r) -> p q r", q=2)[:, :, :, None].to_broadcast([P, 2, 8, 16]),
                    op=OP.mult)
                return sw

            def b_block(g, src_all, dst_all):
                # B invariants for subtile group g: dst[s,l,c] from src[s,i,c]
                sl = slice(g * GRP * 180, (g + 1) * GRP * 180)
                scr = wp.tile([P, GRP * 180], F32, tag="scr")
                nc.scalar.activation(scr[:], src_all[:, sl], AF.Square)
                scr2 = wp.tile([P, GRP * 180], F32, tag="scr2")
                nc.gpsimd.tensor_tensor(
                    out=scr2[:].rearrange("p (g f) -> p g f", f=180),
                    in0=scr[:].rearrange("p (g f) -> p g f", f=180),
                    in1=prefrow[:, None, :].to_broadcast([P, GRP, 180]),
                    op=OP.mult)
                bv = dst_all[:, g * GRP * 45:(g + 1) * GRP * 45].rearrange(
                    "p (s l c) -> p s l c", l=5, c=CHAN)
                sv = scr2[:].rearrange("p (s i c) -> p s i c", i=N_L, c=CHAN)
                av = src_all[:, sl].rearrange("p (s i c) -> p s i c", i=N_L, c=CHAN)
                nc.vector.tensor_copy(bv[:, :, 0, :], av[:, :, 0, :])
                for l, (a, b) in enumerate(L_RANGES):
                    nc.vector.tensor_reduce(
                        out=bv[:, :, l + 1, :],
                        in_=sv[:, :, a:b, :].transpose([0, 1, 3, 2]),
                        axis=mybir.AxisListType.X, op=OP.add)

            # node table in DRAM; AllGather runs as 4 row-group chunks, each
            # writing a contiguous [NC*GRP*SUBN, TABW] block (rank-major)
            tabsh = dr.tile([NROW, TABW], TDT)
            tabfull = dr.tile([NC * NROW, TABW], TDT)
            tabsh_v = tabsh[:].rearrange("(s n) w -> n s w", n=SUBN)
            CHROWS = NC * GRP * SUBN                             # 1024 rows/chunk

            # ---------- stage 1, pipelined per group of 8 subtiles ----------
            s1ctx = tc.tile_pool(name="ps_s1", bufs=3, space="PSUM")
            ps_s1 = s1ctx.__enter__()
            for g in range(NSUB // GRP):
                for s in range(g * GRP, (g + 1) * GRP):
                    t0 = ps_s1.tile([P, 60], F32, space="PSUM", tag="t0")
                    for b2 in range(BPS):
                        blk = s * BPS + b2
                        sw = build_sw1(blk)
                        nc.tensor.matmul(t0[:], lhsT=sw[:], rhs=G1[:, blk * 60:(blk + 1) * 60],
                                         start=(b2 == 0), stop=(b2 == BPS - 1))
                    t0c = wp.tile([P, 60], F32, tag="t0c", bufs=3)
                    nc.scalar.copy(t0c[:], t0[:])
                    t1 = ps_s1.tile([P, 60], F32, space="PSUM", tag="t1")
                    for l, (a, b) in enumerate(L_RANGES):
                        nc.tensor.matmul(t1[:, a * NAB:b * NAB], lhsT=rtl[l][:],
                                         rhs=t0c[:, a * NAB:b * NAB], start=True, stop=True)
                    nc.vector.tensor_tensor(
                        out=A_all[:, s * 180:(s + 1) * 180].rearrange("p (ia b) -> p ia b", b=NAB),
                        in0=t1[:, :, None].to_broadcast([P, 60, NAB]),
                        in1=EM[:, s * NAB:(s + 1) * NAB][:, None, :].to_broadcast([P, 60, NAB]),
                        op=OP.mult)

                # node-level for this group: B0, chi, V
                b_block(g, A_all, B0_all)
                red1 = wp.tile([P, GRP * CHAN], F32, tag="red1")
                nc.vector.tensor_reduce(
                    out=red1[:].rearrange("p (s c) -> p s c", c=CHAN),
                    in_=B0_all[:, g * GRP * 45:(g + 1) * GRP * 45].rearrange(
                        "p (s l c) -> p s c l", l=5, c=CHAN),
                    axis=mybir.AxisListType.X, op=OP.add)
                chips = ps_s1.tile([16, GRP * CHAN], F32, space="PSUM", tag="t0")
                nc.tensor.matmul(chips[:], lhsT=blkdiag[:, 0:16], rhs=red1[:],
                                 start=True, stop=True)
                Vsb = wp.tile([16, GRP * CHAN], TDT, tag="vsb")
                nc.vector.tensor_tensor(
                    out=Vsb[:].rearrange("p (s a b) -> p s a b", a=NAB, b=NAB),
                    in0=chips[:].rearrange("p (s a b) -> p s a b", a=NAB, b=NAB),
                    in1=EM[0:16, :].rearrange("p (s a) -> p s a", a=NAB)[:, g * GRP:(g + 1) * GRP, :, None].to_broadcast([16, GRP, NAB, NAB]),
                    op=OP.mult)
                # memory term for this group (gpsimd; consumed in stage 2)
                nc.gpsimd.tensor_tensor(
                    out=mem_all[:, g * GRP * 180:(g + 1) * GRP * 180].rearrange(
                        "p (s f) -> p s f", f=180),
                    in0=A_all[:, g * GRP * 180:(g + 1) * GRP * 180].rearrange(
                        "p (s f) -> p s f", f=180),
                    in1=WT[:, None, :].to_broadcast([P, GRP, 180]),
                    op=OP.mult)
                # bf16 cast on scalar engine, then HWDGE repack + V columns
                abf = wp.tile([P, GRP * 180], TDT, tag="abf")
                nc.scalar.copy(abf[:], A_all[:, g * GRP * 180:(g + 1) * GRP * 180])
                for sp in range(8):
                    nc.sync.dma_start(
                        out=tabsh_v[:, g * GRP:(g + 1) * GRP, sp * 180:(sp + 1) * 180],
                        in_=abf[sp * 16:(sp + 1) * 16, :].rearrange("n (s f) -> n s f", f=180))
                nc.sync.dma_start(
                    out=tabsh_v[:, g * GRP:(g + 1) * GRP, 1440:1449],
                    in_=Vsb[:].rearrange("n (s c) -> n s c", c=CHAN))
                nc.sync.dma_start(o_b0[:, g * GRP * 45:(g + 1) * GRP * 45],
                                  B0_all[:, g * GRP * 45:(g + 1) * GRP * 45])
                # AllGather this group's rows (sim: local copies model the
                # measured ~17us/1.5MB-rank 8-core AG, scaled per chunk)
                rs = slice(g * GRP * SUBN, (g + 1) * GRP * SUBN)
                if sim_mode:
                    for _cc in range(4):
                        nc.sync.dma_start(
                            tabfull[g * CHROWS + _cc * GRP * SUBN:
                                    g * CHROWS + (_cc + 1) * GRP * SUBN, :],
                            tabsh[rs, :])
                else:
                    nc.gpsimd.collective_compute(
                        "AllGather", mybir.AluOpType.bypass,
                        replica_groups=[list(range(NC))],
                        ins=[tabsh[rs, :]],
                        outs=[tabfull[g * CHROWS:(g + 1) * CHROWS, :]])

            # ---------- stage 2 ----------
            s1ctx.__exit__(None, None, None)
            s2ctx = tc.tile_pool(name="ps_s2", bufs=3, space="PSUM")
            ps_s2 = s2ctx.__enter__()
            GB = 2                         # subtiles per gather call
            for s in range(NSUB):
                if s % GB == 0:
                    gat = wp.tile([P, GB * BPS, TABW], TDT, tag="gat")
                    nc.gpsimd.dma_gather(gat[:], tabfull[:],
                                         gidx[:, s * 16:(s + GB) * 16],
                                         GB * CAP, GB * CAP, TABW)
                t2 = ps_s2.tile([P, 180], F32, space="PSUM", tag="t2")
                a1p = ps_s2.tile([P, 180], F32, space="PSUM", tag="a1p")
                G2 = wp.tile([P, BPS, 180], TDT, tag="g2", bufs=3)
                gbb = (s % GB) * BPS
                nc.vector.tensor_tensor(
                    out=G2[:].rearrange("p b (i c) -> p b i c", c=CHAN),
                    in0=angv[:, s * BPS:(s + 1) * BPS, :][:, :, :, None].to_broadcast([P, BPS, N_L, CHAN]),
                    in1=gat[:, gbb:gbb + BPS, 1440:1449][:, :, None, :].to_broadcast([P, BPS, N_L, CHAN]),
                    op=OP.mult)
                for b2 in range(BPS):
                    blk = s * BPS + b2
                    gb2 = (s % GB) * BPS + b2
                    sw = build_sw(blk, TDT, rcMP)
                    nc.tensor.matmul(t2[:], lhsT=sw[:, 0:128], rhs=G2[:, b2, :],
                                     start=(b2 == 0), stop=False)
                    nc.tensor.matmul(t2[:], lhsT=sw[:, 128:256], rhs=G2[:, b2, :],
                                     start=False, stop=(b2 == BPS - 1))
                    for sig in (0, 2, 4, 6, 1, 3, 5, 7):
                        k, par = sig // 2, sig % 2
                        nc.tensor.matmul(
                            a1p[k * 32:(k + 1) * 32, :],
                            lhsT=sw[:, par * 128 + k * 32: par * 128 + (k + 1) * 32],
                            rhs=gat[:, gb2, sig * 180:(sig + 1) * 180],
                            start=(b2 == 0 and par == 0), stop=False,
                            tile_position=(0, k * 32))
                t2s = wp.tile([P, 180], F32, tag="t2s", bufs=3)
                nc.vector.tensor_tensor(
                    out=t2s[:].rearrange("p (i a b) -> p i a b", a=NAB, b=NAB),
                    in0=t2[:].rearrange("p (i a b) -> p i a b", a=NAB, b=NAB),
                    in1=EM[:, s * NAB:(s + 1) * NAB][:, None, None, :].to_broadcast([P, N_L, NAB, NAB]),
                    op=OP.mult)
                for l, (a, b) in enumerate(L_RANGES):
                    nc.tensor.matmul(a1p[:, a * CHAN:b * CHAN], lhsT=rtl[l][:],
                                     rhs=t2s[:, a * CHAN:b * CHAN], start=False, stop=True)
                nc.vector.tensor_tensor(out=A1_all[:, s * 180:(s + 1) * 180],
                                        in0=a1p[:], in1=mem_all[:, s * 180:(s + 1) * 180],
                                        op=OP.add)
                if s % GRP == GRP - 1:
                    # node-level B1 for the finished group, overlapped with
                    # the remaining subtiles' gathers/matmuls
                    g = s // GRP
                    b_block(g, A1_all, B1_all)
                    nc.sync.dma_start(o_b1[:, g * GRP * 45:(g + 1) * GRP * 45],
                                      B1_all[:, g * GRP * 45:(g + 1) * GRP * 45])

            s2ctx.__exit__(None, None, None)
    nc.compile()
    return nc


# ================= public entry =================
def kernel(positions, shifts, W_emb, W_rt, W_nm, atomic_numbers, edge_index):
    global _PROGRAM
    prep = _prep(positions, shifts, atomic_numbers, edge_index)
    consts, repl16, ones1 = _consts()
    if _PROGRAM is None:
        _PROGRAM = _build()
    nc = _PROGRAM
    wemb = np.asarray(W_emb, dtype=np.float32)
    wrt = np.asarray(W_rt, dtype=np.float32)
    wnm = np.asarray(W_nm, dtype=np.float32)
    # host-replicated weight patterns (pure tiling/gathers of the small weights)
    pg = np.arange(P) // 16                                   # r|s' group per partition
    rtlw = wrt[:, pg, :].transpose(1, 0, 2).reshape(P, 32)    # [p, (l, s')] = W_rt[l, p//16, s']
    wtp = wnm[0, pg][:, L_OF, :].reshape(P, 180)              # [p, (i, c)] = W_nm[0, p//16, l_i, c]
    in_maps = []
    for c in range(NC):
        em = wemb[prep["rowsp"][c].reshape(NSUB, SUBN)]       # [sub, n, a]
        em = em[:, np.arange(P) % 16, :].transpose(1, 0, 2).reshape(P, NSUB * NAB)
        wpack = np.ascontiguousarray(
            np.concatenate([rtlw, wtp, em], axis=1).astype(np.float32))
        embse = np.ascontiguousarray(
            wemb[prep["sendsp"][c]].reshape(P, NBLK * NAB).astype(np.float32))
        in_maps.append(dict(
            x_geo=prep["geo"][c], x_recv=prep["recv"][c], x_gidx=prep["gidx"][c],
            x_cons=consts, x_wpack=wpack, x_embse=embse,
        ))
    res = run_bass_kernel_spmd(nc, in_maps, list(range(NC))).results
    # unshard: [128=(s',n), (sub, l, c)] -> node rows
    out = np.zeros((N_NODES, N_RB, 5, CHAN, 2), dtype=np.float32)
    node_of_row = prep["node_of_row"]
    for c in range(NC):
        for mp, name in ((0, "o_b0"), (1, "o_b1")):
            arr = res[c][name].reshape(8, SUBN, NSUB, 5, CHAN)    # [s', n, sub, l, ch]
            rows = arr.transpose(2, 1, 0, 3, 4).reshape(NROW, N_RB, 5, CHAN)
            valid = node_of_row[c * NROW:(c + 1) * NROW] >= 0
            out[node_of_row[c * NROW:(c + 1) * NROW][valid], :, :, :, mp] = rows[valid]
    return out


# revision 9
# speedup vs baseline: 1.0560x; 1.0560x over previous
"""Trainium2 Bass kernel for the CACE message-passing GNN (nn_Cace_58291296141968).

Strategy (8 NeuronCores, SPMD):
  - Receivers are load-balanced onto 8 cores x 32 subtiles x 16 node slots
    (host-side index prep only). Edges go to the subtile of their receiver,
    padded to 256 edge slots (2 blocks of 128) per subtile.
  - Per-edge radial (bessel*cutoff), angular monomials and species embeddings
    are computed on device in [128, n_blocks*w] layout.
  - Segment sums are PE matmuls: lhsT is a per-block "weighted one-hot"
    S_w[e,(r,n)] = rc[e,r] * delta(recv_slot(e)==n), built on DVE with
    broadcast APs; PSUM accumulates per subtile in layout [(r|s')*16+n, feat].
  - The shared per-l radial transform W_rt is applied post-segsum as 4 const
    block-diagonal matmuls (RTL_l), staying on the partition axis.
  - Stage 1 is pipelined in groups of 8 subtiles: as each group's A lands,
    its B0/chi/V are computed, the group's table rows are cast to bf16 on the
    scalar engine, repacked to DRAM via HWDGE, and AllGathered as a chunked
    collective — overlapping the collective with the rest of stage 1.
  - Message-passing layer: rows of the gathered [4096, 1536] bf16 node table
    [A | V] are fetched per edge with dma_gather; msg_A uses 8 sigma-sliced
    matmuls (parity-split S_w so PSUM writes stay 32-aligned), msg_Bchi uses
    the same seg-matmul + RT with the receiver-embedding factor per node.
  - B0/B1 invariants are computed on device; host only unpermutes rows.
"""
import os
import numpy as np
from math import factorial, pi

import concourse.bacc as bacc
import concourse.bass as bass
import concourse.mybir as mybir
import concourse.tile as tile
from concourse.bass_utils import run_bass_kernel_spmd

# ---- problem constants (hardcoded; must match reference.py) ----
ZS = np.array([1, 6, 7, 8], dtype=np.int64)
NZ = 4
NAB = 3
CHAN = 9
MAX_L = 3
N_RBF = 8
N_RB = 8
CUTOFF = 5.5
MP_NORM = 1.0 / 10.0 ** 0.5
N_NODES = 4000
N_EDGES = 48000

def _make_l_list(max_l):
    lst = []
    for l in range(max_l + 1):
        for lx in range(l, -1, -1):
            for ly in range(l - lx, -1, -1):
                lst.append((lx, ly, l - lx - ly))
    return lst

L_LIST = _make_l_list(MAX_L)
N_L = len(L_LIST)                                   # 20
L_OF = np.array([sum(t) for t in L_LIST])
PREF = np.array([factorial(sum(t)) / (factorial(t[0]) * factorial(t[1]) * factorial(t[2]))
                 for t in L_LIST], dtype=np.float64)
L_RANGES = [(0, 1), (1, 4), (4, 10), (10, 20)]
# monomial build chain: (i, parent, comp) for i >= 1
_MONO_CHAIN = []
for _i in range(1, N_L):
    t = L_LIST[_i]
    for _c in range(3):
        if t[_c] > 0:
            pt = list(t); pt[_c] -= 1
            _MONO_CHAIN.append((_i, L_LIST.index(tuple(pt)), _c))
            break

NC = 8
NSUB = 32
SUBN = 16
BPS = 2
EPB = 128
CAP = BPS * EPB          # 256
NBLK = NSUB * BPS        # 64 blocks/core
NROW = NSUB * SUBN       # 512 node rows/core
TABW = 1536              # table row: 1440 A + 9 V + pad (bytes % 256 == 0)
P = 128
F32 = mybir.dt.float32
BF16 = mybir.dt.bfloat16
TDT = BF16               # table + stage-2 seg dtype
I16 = mybir.dt.int16
GRP = 8                  # subtiles per stage-1/2 pipeline group

_PROGRAM = None


# ================= host-side sharding prep (index work only) =================
def _prep(positions, shifts, atomic_numbers, edge_index):
    import heapq
    snd = np.asarray(edge_index[0]).astype(np.int64)
    rcv = np.asarray(edge_index[1]).astype(np.int64)
    an = np.asarray(atomic_numbers)
    species = np.searchsorted(ZS, an)
    indeg = np.bincount(rcv, minlength=N_NODES)
    order = np.argsort(-indeg, kind="stable")
    TS = NC * NSUB
    loads = np.zeros(TS, dtype=np.int64)
    counts = np.zeros(TS, dtype=np.int64)
    assign_sub = np.zeros(N_NODES, dtype=np.int64)
    assign_slot = np.zeros(N_NODES, dtype=np.int64)
    heap = [(0, t) for t in range(TS)]
    heapq.heapify(heap)
    for nd in order:
        pending = []
        while True:
            load, t = heapq.heappop(heap)
            if counts[t] < SUBN:
                break
            pending.append((load, t))
        assign_sub[nd] = t
        assign_slot[nd] = counts[t]
        counts[t] += 1
        loads[t] = load + indeg[nd]
        heapq.heappush(heap, (loads[t], t))
        for it in pending:
            heapq.heappush(heap, it)
    assert loads.max() <= CAP, f"subtile edge overflow: {loads.max()} > {CAP}"

    core_of = assign_sub // NSUB
    sub_of = assign_sub % NSUB
    node_row = core_of * NROW + sub_of * SUBN + assign_slot      # node -> global row
    node_of_row = np.full(NC * NROW, -1, dtype=np.int64)
    node_of_row[node_row] = np.arange(N_NODES)
    # table row in the chunked-AllGather layout: [group, core, sub%GRP, slot]
    tab_row = ((sub_of // GRP) * (NC * GRP * SUBN) + core_of * (GRP * SUBN)
               + (sub_of % GRP) * SUBN + assign_slot)

    e_sub = assign_sub[rcv]
    e_order = np.argsort(e_sub, kind="stable")
    bounds = np.searchsorted(e_sub[e_order], np.arange(TS + 1))

    pos = np.asarray(positions, dtype=np.float32)
    shf = np.asarray(shifts, dtype=np.float32)

    ES = NSUB * CAP                                              # 8192 edge slots/core
    geo = np.zeros((NC, 9, ES), dtype=np.float32)                # [comp(SxyzRxyzShxyz), slot]
    geo[:, 3:6, :] = 1.0                                         # benign pad: R=(1,1,1), S=0
    recvoh = np.zeros((NC, SUBN, ES), dtype=np.float32)
    sendrow = np.zeros((NC, ES), dtype=np.int64)
    for t in range(TS):
        c = t // NSUB; s = t % NSUB
        es = e_order[bounds[t]:bounds[t + 1]]
        k = len(es)
        base = s * CAP
        geo[c, 0:3, base:base + k] = pos[snd[es]].T
        geo[c, 3:6, base:base + k] = pos[rcv[es]].T
        geo[c, 6:9, base:base + k] = shf[es].T
        recvoh[c, assign_slot[rcv[es]], base + np.arange(k)] = 1.0
        sendrow[c, base:base + k] = tab_row[snd[es]]

    # device edge-slot layout: slot -> (blk, p) with slot = blk*128 + p
    def to_pb(a):   # [NC, ..., ES] -> [NC, 128, ..., NBLK]
        a2 = a.reshape(a.shape[:-1] + (NBLK, EPB))               # [..., NBLK, 128]
        return np.moveaxis(a2, -1, 1)                            # [NC, 128, ..., NBLK]

    geo_in = np.ascontiguousarray(to_pb(geo).reshape(NC, P, 9 * NBLK))   # [NC, 128, (comp,blk)]
    recv_in = np.ascontiguousarray(to_pb(recvoh).reshape(NC, P, SUBN * NBLK))  # [NC,128,(n,blk)]
    # gather idx: per subtile 256 slots; idx k at partition k%16 (replicated), col sub*16 + k//16
    gidx = np.zeros((NC, P, NSUB * 16), dtype=np.int16)
    for c in range(NC):
        w = sendrow[c].reshape(NSUB, 16, 16).astype(np.int16)    # [sub, k//16, k%16]
        packed = w.transpose(2, 0, 1).reshape(16, NSUB * 16)     # [k%16, (sub, k//16)]
        for g in range(8):
            gidx[c, g * 16:(g + 1) * 16, :] = packed
    # per-edge-slot sender species (pad -> 0) in device layout [NC, 128, NBLK]
    sendsp = np.zeros((NC, ES), dtype=np.int64)
    for t in range(TS):
        c = t // NSUB; s = t % NSUB
        es = e_order[bounds[t]:bounds[t + 1]]
        sendsp[c, s * CAP:s * CAP + len(es)] = species[snd[es]]
    sendsp_in = to_pb(sendsp)                                    # [NC, 128, NBLK]
    # per-node-row species (empty rows -> 0; all their uses are masked/zero)
    rowsp = np.zeros((NC, NROW), dtype=np.int64)
    msk = node_of_row >= 0
    rowsp.reshape(-1)[msk] = species[node_of_row[msk]]
    return dict(geo=geo_in, recv=recv_in, gidx=gidx, sendsp=sendsp_in, rowsp=rowsp,
                node_of_row=node_of_row, node_row=node_row)


def _consts():
    iotaN = np.tile((np.arange(P) % 16).astype(np.float32)[None, :], (P, 1))
    blkdiag = ((np.arange(P)[:, None] % 16) == (np.arange(P)[None, :] % 16)).astype(np.float32)
    prefrow = np.tile(np.repeat(PREF.astype(np.float32), CHAN)[None, :], (P, 1))       # [128,180]
    nrow = np.tile((np.arange(1, N_RBF + 1) * pi / CUTOFF).astype(np.float32)[None, :], (P, 1))
    parc = np.zeros((P, 16), dtype=np.float32)                   # [par, r] keep r where r%2==par
    for par in range(2):
        for r in range(8):
            if r % 2 == par:
                parc[:, par * 8 + r] = 1.0
    consts = np.concatenate([iotaN, blkdiag, prefrow, nrow, parc], axis=1)  # [128, 460]
    repl16 = np.zeros((8, P), dtype=np.float32)
    for p in range(P):
        repl16[p // 16, p] = 1.0
    ones1 = np.ones((1, P), dtype=np.float32)
    return consts, repl16, ones1


# ================= device program =================
def _build(sim_mode=False):
    nc = bacc.Bacc("TRN2", target_bir_lowering=False, debug=False,
                   num_devices=(1 if sim_mode else NC))
    AF = mybir.ActivationFunctionType
    OP = mybir.AluOpType

    x_geo = nc.dram_tensor("x_geo", [P, 9 * NBLK], F32, kind="ExternalInput")
    x_recv = nc.dram_tensor("x_recv", [P, SUBN * NBLK], F32, kind="ExternalInput")
    x_gidx = nc.dram_tensor("x_gidx", [P, NSUB * 16], I16, kind="ExternalInput")
    x_cons = nc.dram_tensor("x_cons", [P, 460], F32, kind="ExternalInput")
    # host-replicated weight patterns: [RTLW 32 | WT 180 | EM 96] + per-edge sender emb
    x_wpack = nc.dram_tensor("x_wpack", [P, 308], F32, kind="ExternalInput")
    x_embse = nc.dram_tensor("x_embse", [P, NBLK * NAB], F32, kind="ExternalInput")
    o_b0 = nc.dram_tensor("o_b0", [P, NSUB * 45], F32, kind="ExternalOutput")
    o_b1 = nc.dram_tensor("o_b1", [P, NSUB * 45], F32, kind="ExternalOutput")

    with tile.TileContext(nc) as tc:
        with (
            tc.tile_pool(name="persist", bufs=1) as pp,
            tc.tile_pool(name="work", bufs=2) as wp,
            tc.tile_pool(name="dram", bufs=1, space="DRAM") as dr,
        ):
            # ---------- loads ----------
            cons = pp.tile([P, 460], F32)
            nc.sync.dma_start(cons[:], x_cons[:])
            iotaN = cons[:, 0:128]
            blkdiag = cons[:, 128:256]
            prefrow = cons[:, 256:436]
            nrow = cons[:, 436:444]
            parc = cons[:, 444:460]

            geo = pp.tile([P, 9 * NBLK], F32)
            recvs = pp.tile([P, SUBN * NBLK], F32)
            gidx = pp.tile([P, NSUB * 16], I16)
            wpack = pp.tile([P, 308], F32)
            embsE = pp.tile([P, NBLK * NAB], F32)
            nc.sync.dma_start(geo[:], x_geo[:])
            nc.sync.dma_start(recvs[:], x_recv[:])
            nc.sync.dma_start(gidx[:], x_gidx[:])
            nc.sync.dma_start(wpack[:], x_wpack[:])
            nc.sync.dma_start(embsE[:], x_embse[:])
            WT = wpack[:, 32:212]
            EM = wpack[:, 212:308]

            # ---------- one-time derived: RTL_l from host-shipped W rows ----------
            rtl = []
            for l in range(MAX_L + 1):
                rtl_t = pp.tile([P, P], F32, tag=f"rtl{l}")
                rtl.append(rtl_t)
                nc.vector.tensor_tensor(
                    out=rtl_t[:].rearrange("p (s n) -> p s n", s=8),
                    in0=wpack[:, l * 8:(l + 1) * 8][:, :, None].to_broadcast([P, 8, 16]),
                    in1=blkdiag.rearrange("p (s n) -> p s n", s=8),
                    op=OP.mult)

            # ---------- per-edge base phase ----------
            D = pp.tile([P, 3 * NBLK], F32)
            nc.vector.tensor_tensor(out=D[:], in0=geo[:, 3 * NBLK:6 * NBLK],
                                    in1=geo[:, 0:3 * NBLK], op=OP.subtract)
            nc.vector.tensor_tensor(out=D[:], in0=D[:], in1=geo[:, 6 * NBLK:9 * NBLK], op=OP.add)
            sq = wp.tile([P, 3 * NBLK], F32, tag="sq")
            nc.vector.tensor_tensor(out=sq[:], in0=D[:], in1=D[:], op=OP.mult)
            r2 = wp.tile([P, NBLK], F32, tag="r2")
            nc.vector.tensor_tensor(out=r2[:], in0=sq[:, 0:NBLK], in1=sq[:, NBLK:2 * NBLK], op=OP.add)
            nc.vector.tensor_tensor(out=r2[:], in0=r2[:], in1=sq[:, 2 * NBLK:3 * NBLK], op=OP.add)
            rr = wp.tile([P, NBLK], F32, tag="rr")
            nc.scalar.activation(rr[:], r2[:], AF.Sqrt)
            rinv = pp.tile([P, NBLK], F32)
            nc.vector.reciprocal(rinv[:], rr[:])
            uu = wp.tile([P, NBLK], F32, tag="uu")
            nc.vector.tensor_scalar_mul(uu[:], rr[:], 1.0 / CUTOFF)
            U = pp.tile([P, 3 * NBLK], F32)
            nc.vector.tensor_tensor(
                out=U[:].rearrange("p (c b) -> p c b", c=3),
                in0=D[:].rearrange("p (c b) -> p c b", c=3),
                in1=rinv[:, None, :].to_broadcast([P, 3, NBLK]), op=OP.mult)
            # bessel args [128, (blk, r)] + range reduction to [-pi, pi)
            arg = wp.tile([P, NBLK * 8], F32, tag="arg")
            nc.vector.tensor_tensor(
                out=arg[:].rearrange("p (b r) -> p b r", r=8),
                in0=rr[:, :, None].to_broadcast([P, NBLK, 8]),
                in1=nrow[:, None, :].to_broadcast([P, NBLK, 8]), op=OP.mult)
            ge = wp.tile([P, NBLK * 8], F32, tag="ge")
            for thr, sub in ((4 * pi, 4 * pi), (2 * pi, 2 * pi), (pi, 2 * pi)):
                nc.vector.tensor_scalar(out=ge[:], in0=arg[:], scalar1=float(thr),
                                        scalar2=float(sub), op0=OP.is_ge, op1=OP.mult)
                nc.vector.tensor_tensor(out=arg[:], in0=arg[:], in1=ge[:], op=OP.subtract)
            sinv = wp.tile([P, NBLK * 8], F32, tag="sinv")
            nc.scalar.activation(sinv[:], arg[:], AF.Sin)
            # cutoff polynomial
            u2 = wp.tile([P, NBLK], F32, tag="u2")
            nc.vector.tensor_tensor(out=u2[:], in0=uu[:], in1=uu[:], op=OP.mult)
            a1 = wp.tile([P, NBLK], F32, tag="a1")
            nc.vector.tensor_scalar(out=a1[:], in0=uu[:], scalar1=-48.0, scalar2=28.0,
                                    op0=OP.mult, op1=OP.add)
            g21 = wp.tile([P, NBLK], F32, tag="g21")
            nc.vector.tensor_scalar_mul(g21[:], u2[:], 21.0)
            nc.vector.tensor_tensor(out=g21[:], in0=g21[:], in1=a1[:], op=OP.add)
            u6 = wp.tile([P, NBLK], F32, tag="u6")
            nc.vector.tensor_tensor(out=u6[:], in0=u2[:], in1=u2[:], op=OP.mult)
            nc.vector.tensor_tensor(out=u6[:], in0=u6[:], in1=u2[:], op=OP.mult)
            fc = wp.tile([P, NBLK], F32, tag="fc")
            nc.vector.tensor_tensor(out=fc[:], in0=u6[:], in1=g21[:], op=OP.mult)
            nc.vector.tensor_scalar(out=fc[:], in0=fc[:], scalar1=-1.0, scalar2=1.0,
                                    op0=OP.mult, op1=OP.add)
            lt = wp.tile([P, NBLK], F32, tag="lt")
            nc.vector.tensor_scalar(out=lt[:], in0=uu[:], scalar1=1.0, scalar2=None, op0=OP.is_lt)
            nc.vector.tensor_tensor(out=fc[:], in0=fc[:], in1=lt[:], op=OP.mult)
            scal = wp.tile([P, NBLK], F32, tag="scal")
            nc.vector.tensor_tensor(out=scal[:], in0=rinv[:], in1=fc[:], op=OP.mult)
            nc.vector.tensor_scalar_mul(scal[:], scal[:], float(np.sqrt(2.0 / CUTOFF)))
            rc = pp.tile([P, NBLK * 8], F32)
            nc.vector.tensor_tensor(
                out=rc[:].rearrange("p (b r) -> p b r", r=8),
                in0=sinv[:].rearrange("p (b r) -> p b r", r=8),
                in1=scal[:, :, None].to_broadcast([P, NBLK, 8]), op=OP.mult)
            # parity-masked rc: rcEO [128, (blk, par, r)]; _mp variant folds MP_NORM
            rcEO = pp.tile([P, NBLK * 16], F32)
            nc.vector.tensor_tensor(
                out=rcEO[:].rearrange("p (b q r) -> p b q r", q=2, r=8),
                in0=rc[:].rearrange("p (b r) -> p b r", r=8)[:, :, None, :].to_broadcast([P, NBLK, 2, 8]),
                in1=parc.rearrange("p (q r) -> p q r", q=2)[:, None, :, :].to_broadcast([P, NBLK, 2, 8]),
                op=OP.mult)
            rcMP = pp.tile([P, NBLK * 16], F32)
            nc.vector.tensor_scalar_mul(rcMP[:], rcEO[:], float(MP_NORM))
            # angular monomials ang [128, (blk, i)]
            ang = pp.tile([P, NBLK * N_L], F32)
            angv = ang[:].rearrange("p (b i) -> p b i", i=N_L)
            nc.vector.tensor_scalar(out=angv[:, :, 0], in0=uu[:], scalar1=0.0, scalar2=1.0,
                                    op0=OP.mult, op1=OP.add)
            for i, par, c in _MONO_CHAIN:
                nc.vector.tensor_tensor(out=angv[:, :, i], in0=angv[:, :, par],
                                        in1=U[:, c * NBLK:(c + 1) * NBLK], op=OP.mult)
            # G1 [128, (blk, i, a)]
            G1 = pp.tile([P, NBLK * N_L * NAB], F32)
            nc.vector.tensor_tensor(
                out=G1[:].rearrange("p (b i a) -> p b i a", i=N_L, a=NAB),
                in0=angv[:, :, :, None].to_broadcast([P, NBLK, N_L, NAB]),
                in1=embsE[:].rearrange("p (b a) -> p b a", a=NAB)[:, :, None, :].to_broadcast([P, NBLK, N_L, NAB]),
                op=OP.mult)

            A_all = pp.tile([P, NSUB * 180], F32)
            A1_all = pp.tile([P, NSUB * 180], F32)
            B0_all = pp.tile([P, NSUB * 45], F32)
            B1_all = pp.tile([P, NSUB * 45], F32)
            mem_all = pp.tile([P, NSUB * 180], F32)

            def build_sw1(blk):
                # unsplit f32 S_w for stage 1: [128, (r,n)]
                sw = wp.tile([P, P], F32, tag="sw1", bufs=3)
                nc.vector.tensor_tensor(
                    out=sw[:].rearrange("p (r n) -> p r n", r=8),
                    in0=recvs[:].rearrange("p (n b) -> p b n", n=SUBN)[:, blk, :][:, None, :].to_broadcast([P, 8, 16]),
                    in1=rc[:, blk * 8:(blk + 1) * 8][:, :, None].to_broadcast([P, 8, 16]),
                    op=OP.mult)
                return sw

            def build_sw(blk, dt, rcsrc):
                sw = wp.tile([P, 256], dt, tag="sw" if dt == F32 else "swb", bufs=3)
                nc.vector.tensor_tensor(
                    out=sw[:].rearrange("p (q r n) -> p q r n", q=2, r=8),
                    in0=recvs[:].rearrange("p (n b) -> p b n", n=SUBN)[:, blk, :][:, None, None, :].to_broadcast([P, 2, 8, 16]),
                    in1=rcsrc[:, blk * 16:(blk + 1) * 16].rearrange("p (q r) -> p q r", q=2)[:, :, :, None].to_broadcast([P, 2, 8, 16]),
                    op=OP.mult)
                return sw

            def b_block(g, src_all, dst_all):
                # B invariants for subtile group g: dst[s,l,c] from src[s,i,c]
                sl = slice(g * GRP * 180, (g + 1) * GRP * 180)
                scr = wp.tile([P, GRP * 180], F32, tag="scr")
                nc.scalar.activation(scr[:], src_all[:, sl], AF.Square)
                scr2 = wp.tile([P, GRP * 180], F32, tag="scr2")
                nc.gpsimd.tensor_tensor(
                    out=scr2[:].rearrange("p (g f) -> p g f", f=180),
                    in0=scr[:].rearrange("p (g f) -> p g f", f=180),
                    in1=prefrow[:, None, :].to_broadcast([P, GRP, 180]),
                    op=OP.mult)
                bv = dst_all[:, g * GRP * 45:(g + 1) * GRP * 45].rearrange(
                    "p (s l c) -> p s l c", l=5, c=CHAN)
                sv = scr2[:].rearrange("p (s i c) -> p s i c", i=N_L, c=CHAN)
                av = src_all[:, sl].rearrange("p (s i c) -> p s i c", i=N_L, c=CHAN)
                nc.scalar.copy(bv[:, :, 0, :], av[:, :, 0, :])
                for l, (a, b) in enumerate(L_RANGES):
                    nc.vector.tensor_reduce(
                        out=bv[:, :, l + 1, :],
                        in_=sv[:, :, a:b, :].transpose([0, 1, 3, 2]),
                        axis=mybir.AxisListType.X, op=OP.add)

            # node table in DRAM; AllGather runs as 4 row-group chunks, each
            # writing a contiguous [NC*GRP*SUBN, TABW] block (rank-major)
            tabsh = dr.tile([NROW, TABW], TDT)
            tabfull = dr.tile([NC * NROW, TABW], TDT)
            tabsh_v = tabsh[:].rearrange("(s n) w -> n s w", n=SUBN)
            CHROWS = NC * GRP * SUBN                             # 1024 rows/chunk

            # ---------- stage 1, software-pipelined per group of 8 subtiles:
            # group g's node-level work is emitted after group g+1's A loop so
            # the in-order DVE stream never stalls on the Act/Pool B0 chain.
            s1ctx = tc.tile_pool(name="ps_s1", bufs=3, space="PSUM")
            ps_s1 = s1ctx.__enter__()

            def a_loop(g):
                for s in range(g * GRP, (g + 1) * GRP):
                    t0 = ps_s1.tile([P, 60], F32, space="PSUM", tag="t0")
                    for b2 in range(BPS):
                        blk = s * BPS + b2
                        sw = build_sw1(blk)
                        nc.tensor.matmul(t0[:], lhsT=sw[:], rhs=G1[:, blk * 60:(blk + 1) * 60],
                                         start=(b2 == 0), stop=(b2 == BPS - 1))
                    t0c = wp.tile([P, 60], F32, tag="t0c", bufs=3)
                    nc.scalar.copy(t0c[:], t0[:])
                    t1 = ps_s1.tile([P, 60], F32, space="PSUM", tag="t1")
                    for l, (a, b) in enumerate(L_RANGES):
                        nc.tensor.matmul(t1[:, a * NAB:b * NAB], lhsT=rtl[l][:],
                                         rhs=t0c[:, a * NAB:b * NAB], start=True, stop=True)
                    nc.vector.tensor_tensor(
                        out=A_all[:, s * 180:(s + 1) * 180].rearrange("p (ia b) -> p ia b", b=NAB),
                        in0=t1[:, :, None].to_broadcast([P, 60, NAB]),
                        in1=EM[:, s * NAB:(s + 1) * NAB][:, None, :].to_broadcast([P, 60, NAB]),
                        op=OP.mult)

            def node_level(g):
                # B0, chi, V, bf16 repack + AllGather chunk for group g
                b_block(g, A_all, B0_all)
                red1 = wp.tile([P, GRP * CHAN], F32, tag="red1")
                nc.vector.tensor_reduce(
                    out=red1[:].rearrange("p (s c) -> p s c", c=CHAN),
                    in_=B0_all[:, g * GRP * 45:(g + 1) * GRP * 45].rearrange(
                        "p (s l c) -> p s c l", l=5, c=CHAN),
                    axis=mybir.AxisListType.X, op=OP.add)
                chips = ps_s1.tile([16, GRP * CHAN], F32, space="PSUM", tag="t0")
                nc.tensor.matmul(chips[:], lhsT=blkdiag[:, 0:16], rhs=red1[:],
                                 start=True, stop=True)
                Vsb = wp.tile([16, GRP * CHAN], TDT, tag="vsb")
                nc.vector.tensor_tensor(
                    out=Vsb[:].rearrange("p (s a b) -> p s a b", a=NAB, b=NAB),
                    in0=chips[:].rearrange("p (s a b) -> p s a b", a=NAB, b=NAB),
                    in1=EM[0:16, :].rearrange("p (s a) -> p s a", a=NAB)[:, g * GRP:(g + 1) * GRP, :, None].to_broadcast([16, GRP, NAB, NAB]),
                    op=OP.mult)
                # memory term for this group (gpsimd; consumed in stage 2)
                nc.gpsimd.tensor_tensor(
                    out=mem_all[:, g * GRP * 180:(g + 1) * GRP * 180].rearrange(
                        "p (s f) -> p s f", f=180),
                    in0=A_all[:, g * GRP * 180:(g + 1) * GRP * 180].rearrange(
                        "p (s f) -> p s f", f=180),
                    in1=WT[:, None, :].to_broadcast([P, GRP, 180]),
                    op=OP.mult)
                # bf16 cast on scalar engine, then HWDGE repack + V columns
                abf = wp.tile([P, GRP * 180], TDT, tag="abf")
                nc.scalar.copy(abf[:], A_all[:, g * GRP * 180:(g + 1) * GRP * 180])
                for sp in range(8):
                    nc.sync.dma_start(
                        out=tabsh_v[:, g * GRP:(g + 1) * GRP, sp * 180:(sp + 1) * 180],
                        in_=abf[sp * 16:(sp + 1) * 16, :].rearrange("n (s f) -> n s f", f=180))
                nc.sync.dma_start(
                    out=tabsh_v[:, g * GRP:(g + 1) * GRP, 1440:1449],
                    in_=Vsb[:].rearrange("n (s c) -> n s c", c=CHAN))
                nc.sync.dma_start(o_b0[:, g * GRP * 45:(g + 1) * GRP * 45],
                                  B0_all[:, g * GRP * 45:(g + 1) * GRP * 45])
                # AllGather this group's rows (sim: local copies model the
                # measured ~17us/1.5MB-rank 8-core AG, scaled per chunk)
                rs = slice(g * GRP * SUBN, (g + 1) * GRP * SUBN)
                if sim_mode:
                    for _cc in range(4):
                        nc.sync.dma_start(
                            tabfull[g * CHROWS + _cc * GRP * SUBN:
                                    g * CHROWS + (_cc + 1) * GRP * SUBN, :],
                            tabsh[rs, :])
                else:
                    nc.gpsimd.collective_compute(
                        "AllGather", mybir.AluOpType.bypass,
                        replica_groups=[list(range(NC))],
                        ins=[tabsh[rs, :]],
                        outs=[tabfull[g * CHROWS:(g + 1) * CHROWS, :]])

            NG = NSUB // GRP
            a_loop(0)
            for g in range(NG):
                if g + 1 < NG:
                    a_loop(g + 1)
                node_level(g)

            # ---------- stage 2 ----------
            s1ctx.__exit__(None, None, None)
            s2ctx = tc.tile_pool(name="ps_s2", bufs=3, space="PSUM")
            ps_s2 = s2ctx.__enter__()
            GB = 2                         # subtiles per gather call
            for s in range(NSUB):
                if s % GB == 0:
                    gat = wp.tile([P, GB * BPS, TABW], TDT, tag="gat")
                    nc.gpsimd.dma_gather(gat[:], tabfull[:],
                                         gidx[:, s * 16:(s + GB) * 16],
                                         GB * CAP, GB * CAP, TABW)
                t2 = ps_s2.tile([P, 180], F32, space="PSUM", tag="t2")
                a1p = ps_s2.tile([P, 180], F32, space="PSUM", tag="a1p")
                G2 = wp.tile([P, BPS, 180], TDT, tag="g2", bufs=3)
                gbb = (s % GB) * BPS
                nc.vector.tensor_tensor(
                    out=G2[:].rearrange("p b (i c) -> p b i c", c=CHAN),
                    in0=angv[:, s * BPS:(s + 1) * BPS, :][:, :, :, None].to_broadcast([P, BPS, N_L, CHAN]),
                    in1=gat[:, gbb:gbb + BPS, 1440:1449][:, :, None, :].to_broadcast([P, BPS, N_L, CHAN]),
                    op=OP.mult)
                for b2 in range(BPS):
                    blk = s * BPS + b2
                    gb2 = (s % GB) * BPS + b2
                    sw = build_sw(blk, TDT, rcMP)
                    nc.tensor.matmul(t2[:], lhsT=sw[:, 0:128], rhs=G2[:, b2, :],
                                     start=(b2 == 0), stop=False)
                    nc.tensor.matmul(t2[:], lhsT=sw[:, 128:256], rhs=G2[:, b2, :],
                                     start=False, stop=(b2 == BPS - 1))
                    for sig in (0, 2, 4, 6, 1, 3, 5, 7):
                        k, par = sig // 2, sig % 2
                        nc.tensor.matmul(
                            a1p[k * 32:(k + 1) * 32, :],
                            lhsT=sw[:, par * 128 + k * 32: par * 128 + (k + 1) * 32],
                            rhs=gat[:, gb2, sig * 180:(sig + 1) * 180],
                            start=(b2 == 0 and par == 0), stop=False,
                            tile_position=(0, k * 32))
                t2s = wp.tile([P, 180], F32, tag="t2s", bufs=3)
                nc.vector.tensor_tensor(
                    out=t2s[:].rearrange("p (i a b) -> p i a b", a=NAB, b=NAB),
                    in0=t2[:].rearrange("p (i a b) -> p i a b", a=NAB, b=NAB),
                    in1=EM[:, s * NAB:(s + 1) * NAB][:, None, None, :].to_broadcast([P, N_L, NAB, NAB]),
                    op=OP.mult)
                for l, (a, b) in enumerate(L_RANGES):
                    nc.tensor.matmul(a1p[:, a * CHAN:b * CHAN], lhsT=rtl[l][:],
                                     rhs=t2s[:, a * CHAN:b * CHAN], start=False, stop=True)
                nc.vector.tensor_tensor(out=A1_all[:, s * 180:(s + 1) * 180],
                                        in0=a1p[:], in1=mem_all[:, s * 180:(s + 1) * 180],
                                        op=OP.add)
                if s % GRP == GRP - 1:
                    # node-level B1 for the finished group, overlapped with
                    # the remaining subtiles' gathers/matmuls
                    g = s // GRP
                    b_block(g, A1_all, B1_all)
                    nc.sync.dma_start(o_b1[:, g * GRP * 45:(g + 1) * GRP * 45],
                                      B1_all[:, g * GRP * 45:(g + 1) * GRP * 45])

            s2ctx.__exit__(None, None, None)
    nc.compile()
    return nc


# ================= public entry =================
def kernel(positions, shifts, W_emb, W_rt, W_nm, atomic_numbers, edge_index):
    global _PROGRAM
    prep = _prep(positions, shifts, atomic_numbers, edge_index)
    consts, repl16, ones1 = _consts()
    if _PROGRAM is None:
        _PROGRAM = _build()
    nc = _PROGRAM
    wemb = np.asarray(W_emb, dtype=np.float32)
    wrt = np.asarray(W_rt, dtype=np.float32)
    wnm = np.asarray(W_nm, dtype=np.float32)
    # host-replicated weight patterns (pure tiling/gathers of the small weights)
    pg = np.arange(P) // 16                                   # r|s' group per partition
    rtlw = wrt[:, pg, :].transpose(1, 0, 2).reshape(P, 32)    # [p, (l, s')] = W_rt[l, p//16, s']
    wtp = wnm[0, pg][:, L_OF, :].reshape(P, 180)              # [p, (i, c)] = W_nm[0, p//16, l_i, c]
    in_maps = []
    for c in range(NC):
        em = wemb[prep["rowsp"][c].reshape(NSUB, SUBN)]       # [sub, n, a]
        em = em[:, np.arange(P) % 16, :].transpose(1, 0, 2).reshape(P, NSUB * NAB)
        wpack = np.ascontiguousarray(
            np.concatenate([rtlw, wtp, em], axis=1).astype(np.float32))
        embse = np.ascontiguousarray(
            wemb[prep["sendsp"][c]].reshape(P, NBLK * NAB).astype(np.float32))
        in_maps.append(dict(
            x_geo=prep["geo"][c], x_recv=prep["recv"][c], x_gidx=prep["gidx"][c],
            x_cons=consts, x_wpack=wpack, x_embse=embse,
        ))
    res = run_bass_kernel_spmd(nc, in_maps, list(range(NC))).results
    # unshard: [128=(s',n), (sub, l, c)] -> node rows
    out = np.zeros((N_NODES, N_RB, 5, CHAN, 2), dtype=np.float32)
    node_of_row = prep["node_of_row"]
    for c in range(NC):
        for mp, name in ((0, "o_b0"), (1, "o_b1")):
            arr = res[c][name].reshape(8, SUBN, NSUB, 5, CHAN)    # [s', n, sub, l, ch]
            rows = arr.transpose(2, 1, 0, 3, 4).reshape(NROW, N_RB, 5, CHAN)
            valid = node_of_row[c * NROW:(c + 1) * NROW] >= 0
            out[node_of_row[c * NROW:(c + 1) * NROW][valid], :, :, :, mp] = rows[valid]
    return out
